# revision 43
# baseline (speedup 1.0000x reference)
"""Trainium2 Bass kernel for a dense transformer block (nn_Block_25366076850386).

Sharding (8 cores): core c -> batch b = c//2, head-half hh = c%2.
Each core computes LN1+QKV+attention for its 8 heads over its full batch
row, AllGathers attention outputs (bf16) within the (2b, 2b+1) pair,
computes the full attention projection + residual, then FFN with the FF
dim split in half per core. Host sums the pair's partial outputs:
    out[b] = part[2b] + part[2b+1],  part = 0.5*r1 + ffn_half(r1)

v2: all GEMM operands in bf16 (tolerance 2e-2 allows it), core weights
resident in SBUF (loaded once), FFN weights streamed per chunk in
halves/quarters, chunk-pipelined emission so the AllGather overlaps the
next chunk's attention, batched softmax reciprocals, causal mask via
gpsimd affine_select on the exp() output (keeps DVE free), PE transposes
batched 4-per-PSUM-bank with one strided evacuation copy.
"""

import numpy as np
import ml_dtypes

import concourse.bass as bass
import concourse.mybir as mybir
from concourse import bacc
from concourse.tile import TileContext
from concourse.masks import make_identity
from concourse.bass_utils import run_bass_kernel_spmd

F32 = mybir.dt.float32
BF16 = mybir.dt.bfloat16
AF = mybir.ActivationFunctionType
ALU = mybir.AluOpType

B, T, C, H, D, FF = 4, 2048, 1024, 16, 64, 4096
HPC = H // 2          # heads per core = 8
FQ = HPC * D          # per-core q/k/v width = 512
FFH = FF // 2         # per-core FF width = 2048
NT = T // 128         # 16 token tiles
NCT = C // 128        # 8 channel tiles
NCH = T // 512        # 4 token chunks (512 each)
EPS = 1e-5

_CACHED = {}
DEBUG_DUMP = False


def _build_program(has_bqk: bool, has_bv: bool, has_bfc: bool, reps: int = 1):
    nc = bacc.Bacc()

    xin = nc.dram_tensor("xin", [T, C], F32, kind="ExternalInput")
    wqk = nc.dram_tensor("wqk", [C, 2 * FQ], BF16, kind="ExternalInput")
    wvt = nc.dram_tensor("wvt", [C, FQ], BF16, kind="ExternalInput")
    wpt = nc.dram_tensor("wpt", [C, C], BF16, kind="ExternalInput")
    wfct = nc.dram_tensor("wfct", [C, FFH], BF16, kind="ExternalInput")
    wfpt = nc.dram_tensor("wfpt", [FFH, C], BF16, kind="ExternalInput")
    out = nc.dram_tensor("out", [T, C], BF16, kind="ExternalOutput")
    dbg = {}
    if DEBUG_DUMP:
        for nm, shp in [("d_h1T", [128, NCT * 512]), ("d_qT", [128, 4 * 512]),
                        ("d_kT", [128, 4 * 512]),
                        ("d_vON", [128, 4 * HPC * (D + 1)]),
                        ("d_et", [128, 512]), ("d_ysb", [64, 512]),
                        ("d_rec", [1, 512]), ("d_ycon", [128, 4 * 512]),
                        ("d_yf", [128, 8 * 512]), ("d_r1", [128, C]),
                        ("d_g", [128, 16 * 512])]:
            dbg[nm] = nc.dram_tensor(nm, shp, BF16 if nm != "d_rec" else F32,
                                     kind="ExternalOutput")
    bqk_d = bv_d = bfc_d = None
    if has_bqk:
        bqk_d = nc.dram_tensor("bqk", [2, FQ], F32, kind="ExternalInput")
    if has_bv:
        bv_d = nc.dram_tensor("bv", [FQ], F32, kind="ExternalInput")
    if has_bfc:
        bfc_d = nc.dram_tensor("bfc", [FFH], F32, kind="ExternalInput")

    x_t = xin[:].rearrange("(nt p) c -> nt p c", p=128)
    wqk_r = wqk[:].rearrange("(ct p) f -> ct p f", p=128)
    wvt_r = wvt[:].rearrange("(ct p) f -> ct p f", p=128)
    wpt_r = wpt[:].rearrange("(g p) o -> g p o", p=128)
    wfc_v = wfct[:].rearrange("(ct p) f -> p ct f", p=128)    # [128, 8, 2048]
    wfp_v = wfpt[:].rearrange("(ft p) c -> p ft c", p=128)    # [128, 16, 1024]
    out_t = out[:].rearrange("(nt p) c -> nt p c", p=128)

    from contextlib import ExitStack
    with TileContext(nc) as tc:
        with ExitStack() as stack:
            def pool(name, bufs, space="SBUF"):
                return stack.enter_context(
                    tc.tile_pool(name=name, bufs=bufs, space=space))
            persist = pool("persist", 1)
            kvp = pool("kv", 1)
            lnp = pool("lnp", 2)
            xp = pool("xp", 2)
            xcp = pool("xcp", 4)
            qtp = pool("qtp", 1)
            htp = pool("htp", 2)
            etp = pool("etp", 4)
            ysp = pool("ysp", 5)
            dnp = pool("dnp", 1)
            ycp = pool("ycp", 1)
            yfp = pool("yfp", 1)
            x2p = pool("x2p", 2)
            r1p = pool("r1p", 4)
            gp = pool("gp", 1)
            otp = pool("otp", 4)
            wfcs = pool("wfcs", 2)
            wfps = pool("wfps", 1)
            ps_a = pool("ps_a", 2, "PSUM")
            ps_s = pool("ps_s", 2, "PSUM")
            ps_y = pool("ps_y", 2, "PSUM")
            ps_f = pool("ps_f", 2, "PSUM")
            dram = pool("dram", 1, "DRAM")
            # ---------- constants ----------
            ident = persist.tile([128, 128], BF16, tag="ident")
            make_identity(nc, ident)
            eps_sb = persist.tile([128, 1], F32, tag="eps")
            nc.vector.memset(eps_sb, EPS)
            ones64 = persist.tile([1, 64], F32, tag="ones64")
            nc.vector.memset(ones64, 1.0)

            # ---------- resident weights (bf16) ----------
            wqk_sb = persist.tile([128, NCT, 2 * FQ], BF16, tag="wqk")
            wv_sb = persist.tile([128, NCT, FQ], BF16, tag="wv")
            wp_sb = persist.tile([128, NCT, C], BF16, tag="wp")
            for ct in range(NCT):
                nc.sync.dma_start(wqk_sb[:, ct, :], wqk_r[ct])
                nc.sync.dma_start(wv_sb[:, ct, :], wvt_r[ct])
                nc.sync.dma_start(wp_sb[:, ct, :], wpt_r[ct])
            bqk_sb = bv_sb = bfc_sb = None
            if has_bqk:
                bqk_sb = persist.tile([128, 2, FQ // 128], F32, tag="bqk")
                nc.sync.dma_start(
                    bqk_sb, bqk_d[:].rearrange("q (g p) -> p q g", p=128))
            if has_bv:
                bv_sb = persist.tile([128, FQ // 128], F32, tag="bv")
                nc.sync.dma_start(
                    bv_sb, bv_d[:].rearrange("(g p) -> p g", p=128))
            if has_bfc:
                bfc_sb = persist.tile([128, FFH // 128], F32, tag="bfc")
                nc.sync.dma_start(
                    bfc_sb, bfc_d[:].rearrange("(g p) -> p g", p=128))

            def emit_block(rep_i):
                # persistent K^T [128(2h*64d), 4g, T] and V(+ones) tiles
                kT = kvp.tile([128, 4, T], BF16, tag="kT", name=f"kT{rep_i}")
                vON = kvp.tile([128, NT, HPC, D + 1], BF16, tag="vON",
                               name=f"vON{rep_i}")
                nc.gpsimd.memset(vON, 1.0)   # ones column; data cols redone
                agos = [None] * NCH
                r1_of = [None] * NCH
                ln_state = {}

                def layernorm_transpose(j, src_tiles, hT, mv4, fused_rstd):
                    """Normalize 4 token tiles of chunk j and transpose into
                    hT [128, NCT, 512].  src_tiles: 4 aps [128, C] (x fp32 for
                    LN1 two-pass; r1 bf16 for LN2 fused)."""
                    sd4 = lnp.tile([128, 4], F32, tag="sd4")
                    nc.scalar.activation(sd4, mv4[:, :, 1], AF.Sqrt,
                                         bias=eps_sb)
                    rstd4 = lnp.tile([128, 4], F32, tag="rstd4")
                    nc.vector.reciprocal(rstd4, sd4)
                    for tsub in range(4):
                        if fused_rstd:
                            h1 = xcp.tile([128, C], BF16, tag="xc")
                            nc.vector.tensor_scalar(
                                out=h1, in0=src_tiles[tsub],
                                scalar1=mv4[:, tsub, 0:1],
                                scalar2=rstd4[:, tsub:tsub + 1],
                                op0=ALU.subtract, op1=ALU.mult)
                        else:
                            h1 = src_tiles[tsub]
                            nc.vector.tensor_scalar(
                                out=h1, in0=h1,
                                scalar1=rstd4[:, tsub:tsub + 1], scalar2=None,
                                op0=ALU.mult)
                        tp = ps_a.tile([128, 1024], BF16, tag="ps")
                        for ct in range(NCT):
                            nc.tensor.matmul(
                                tp[:, ct * 128:(ct + 1) * 128],
                                h1[:, ct * 128:(ct + 1) * 128], ident,
                                is_transpose=True, skip_group_check=True)
                        nc.vector.tensor_copy(
                            hT[:, :, tsub * 128:(tsub + 1) * 128],
                            tp[:].rearrange("p (q f) -> p q f", q=NCT))

                def phaseA_pre(j):
                    # ---- LN1 (two-pass: xc = x - mu in bf16, then *rstd)
                    mv4 = lnp.tile([128, 4, 2], F32, tag="mv4")
                    xcs = []
                    for tsub in range(4):
                        it = j * 4 + tsub
                        xt = xp.tile([128, C], F32, tag="x")
                        nc.sync.dma_start(xt, x_t[it])
                        stats = lnp.tile([128, 2, 6], F32, tag="stats")
                        nc.vector.bn_stats(stats[:, 0, :], xt[:, 0:512])
                        nc.vector.bn_stats(stats[:, 1, :], xt[:, 512:1024])
                        nc.vector.bn_aggr(mv4[:, tsub, :], stats)
                        xc = xcp.tile([128, C], BF16, tag="xc")
                        nc.vector.tensor_scalar(
                            out=xc, in0=xt, scalar1=mv4[:, tsub, 0:1],
                            scalar2=None, op0=ALU.subtract)
                        xcs.append(xc)
                    h1T = htp.tile([128, NCT, 512], BF16, tag="hT",
                                   name=f"h1T{rep_i}_{j}")
                    layernorm_transpose(j, xcs, h1T, mv4, fused_rstd=False)
                    if DEBUG_DUMP and j == 0:
                        nc.sync.dma_start(
                            dbg["d_h1T"][:].rearrange(
                                "p (a b) -> p a b", a=NCT), h1T)

                    # ---- Q,K projections: out [f(128), 512t] per g
                    qT = qtp.tile([128, 4, 512], BF16, tag="qT",
                                  name=f"qT{rep_i}_{j}")
                    for g in range(8):
                        ps = ps_a.tile([128, 512], F32, tag="ps")
                        for ct in range(NCT):
                            nc.tensor.matmul(
                                ps, wqk_sb[:, ct, g * 128:(g + 1) * 128],
                                h1T[:, ct, :], start=(ct == 0),
                                stop=(ct == NCT - 1), skip_group_check=True)
                        if g < 4:
                            dst = qT[:, g, :]
                        else:
                            dst = kT[:, g - 4, j * 512:(j + 1) * 512]
                        if has_bqk:
                            nc.vector.tensor_scalar_add(
                                out=dst, in0=ps,
                                scalar1=bqk_sb[:, 0 if g < 4 else 1,
                                               g % 4:g % 4 + 1])
                        else:
                            nc.vector.tensor_copy(dst, ps)

                    if DEBUG_DUMP and j == 0:
                        nc.sync.dma_start(
                            dbg["d_qT"][:].rearrange(
                                "p (a b) -> p a b", a=4), qT)
                        nc.sync.dma_start(
                            dbg["d_kT"][:].rearrange(
                                "p (a b) -> p a b", a=4), kT[:, :, 0:512])
                    # ---- V projection: out [128t, 512f] per t-tile
                    for tsub in range(4):
                        it = j * 4 + tsub
                        ps = ps_a.tile([128, 512], F32, tag="ps")
                        for ct in range(NCT):
                            nc.tensor.matmul(
                                ps, h1T[:, ct, tsub * 128:(tsub + 1) * 128],
                                wv_sb[:, ct, :], start=(ct == 0),
                                stop=(ct == NCT - 1), skip_group_check=True)
                        nc.vector.tensor_copy(
                            vON[:, it, :, 0:D],
                            ps.rearrange("p (h d) -> p h d", h=HPC))

                    if DEBUG_DUMP and j == 0:
                        nc.sync.dma_start(
                            dbg["d_vON"][:].rearrange(
                                "p (a b c) -> p a b c", a=4, b=HPC),
                            vON[:, 0:4, :, :])
                    ycon = ycp.tile([128, 4, 512], BF16, tag="ycon",
                                    name=f"ycon{rep_i}_{j}")
                    return qT, ycon

                def attention_head(j, h, qT, ycon):
                    nkt = 4 * j + 4
                    g, poff = h // 2, (h % 2) * 64
                    yps = ps_y.tile([65, 512], F32, tag="y")
                    for kt in range(nkt):
                        sps = ps_s.tile([128, 512], F32, tag="s")
                        nc.tensor.matmul(
                            sps,
                            kT[poff:poff + 64, g, kt * 128:(kt + 1) * 128],
                            qT[poff:poff + 64, g, :],
                            start=True, stop=True, skip_group_check=True)
                        et = etp.tile([128, 512], BF16, tag="et")
                        nc.scalar.activation(et, sps, AF.Exp)
                        if kt >= 4 * j:
                            # zero the strictly-above-diagonal part
                            nc.gpsimd.affine_select(
                                out=et, in_=et, compare_op=ALU.is_ge,
                                fill=0.0, base=-128 * (kt - 4 * j),
                                pattern=[[1, 512]], channel_multiplier=-1)
                        if DEBUG_DUMP and j == 0 and h == 0 and kt == 0:
                            nc.sync.dma_start(dbg["d_et"][:], et)
                        nc.tensor.matmul(
                            yps, vON[:, kt, h, :], et, start=(kt == 0),
                            stop=(kt == nkt - 1), skip_group_check=True)
                    ysb = ysp.tile([64, 512], BF16, tag="ysb")
                    nc.vector.tensor_copy(ysb, yps[0:64, :])
                    den = dnp.tile([1, 512], F32, tag="den")
                    nc.scalar.activation(den, yps[64:65, :], AF.Copy)
                    rec = dnp.tile([1, 512], F32, tag="recf")
                    nc.vector.reciprocal_approx_fast(rec, den)
                    bcps = ps_a.tile([64, 512], F32, tag="ps")
                    nc.tensor.matmul(bcps, ones64, rec,
                                     start=True, stop=True,
                                     skip_group_check=True)
                    if DEBUG_DUMP and j == 0 and h == 0:
                        nc.sync.dma_start(dbg["d_ysb"][:], ysb)
                        nc.sync.dma_start(dbg["d_rec"][:], rec)
                    nc.vector.tensor_tensor(
                        out=ycon[poff:poff + 64, g, :],
                        in0=ysb, in1=bcps, op=ALU.mult)
                    if has_bv:
                        nc.vector.tensor_scalar_add(
                            out=ycon[poff:poff + 64, g, :],
                            in0=ycon[poff:poff + 64, g, :],
                            scalar1=bv_sb[poff:poff + 64, g:g + 1])

                def allgather(j, ycon):
                    if DEBUG_DUMP and j == 0:
                        nc.sync.dma_start(
                            dbg["d_ycon"][:].rearrange(
                                "p (a b) -> p a b", a=4), ycon)
                    agi = dram.tile([FQ, 512], BF16, tag=f"agi{rep_i}_{j}",
                                    name=f"agi{rep_i}_{j}")
                    nc.sync.dma_start(
                        agi[:].rearrange("(g p) q -> p g q", p=128), ycon)
                    ago = dram.tile([2 * FQ, 512], BF16,
                                    tag=f"ago{rep_i}_{j}",
                                    name=f"ago{rep_i}_{j}")
                    nc.gpsimd.collective_compute(
                        "AllGather", ALU.bypass,
                        replica_groups=[[0, 1], [2, 3], [4, 5], [6, 7]],
                        ins=[agi[:]], outs=[ago[:]])
                    agos[j] = ago

                def phaseB_pieces(j):
                    """FFN of chunk j as a list of closures for zipping."""
                    st = {}

                    def proj_tsub(tsub):
                        it = j * 4 + tsub
                        x2 = x2p.tile([128, C], F32, tag="x2")
                        nc.sync.dma_start(x2, x_t[it])
                        r1 = r1p.tile([128, C], BF16, tag="r1")
                        for nchk in range(2):
                            zps = ps_f.tile([128, 512], F32, tag="f")
                            for g8 in range(8):
                                nc.tensor.matmul(
                                    zps,
                                    st["yf"][:, g8,
                                             tsub * 128:(tsub + 1) * 128],
                                    wp_sb[:, g8, nchk * 512:(nchk + 1) * 512],
                                    start=(g8 == 0), stop=(g8 == 7),
                                    skip_group_check=True)
                            nc.vector.tensor_tensor(
                                out=r1[:, nchk * 512:(nchk + 1) * 512],
                                in0=zps,
                                in1=x2[:, nchk * 512:(nchk + 1) * 512],
                                op=ALU.add)
                        stats = lnp.tile([128, 2, 6], F32, tag="stats")
                        nc.vector.bn_stats(stats[:, 0, :], r1[:, 0:512])
                        nc.vector.bn_stats(stats[:, 1, :], r1[:, 512:1024])
                        nc.vector.bn_aggr(st["mv4b"][:, tsub, :], stats)
                        st["r1s"].append(r1)

                    def p0():
                        yf = yfp.tile([128, 8, 512], BF16, tag="yf",
                                      name=f"yf{rep_i}_{j}")
                        nc.sync.dma_start(
                            yf,
                            agos[j][:].rearrange("(g p) q -> p g q", p=128))
                        st["yf"] = yf
                        st["mv4b"] = lnp.tile([128, 4, 2], F32, tag="mv4",
                                      name=f"mv4b{rep_i}_{j}")
                        st["r1s"] = []
                        proj_tsub(0)
                        proj_tsub(1)

                    def p1():
                        proj_tsub(2)
                        proj_tsub(3)
                        r1_of[j] = st["r1s"]
                        if DEBUG_DUMP and j == 0:
                            nc.sync.dma_start(
                                dbg["d_yf"][:].rearrange(
                                    "p (a b) -> p a b", a=8), st["yf"])
                            nc.sync.dma_start(dbg["d_r1"][:], st["r1s"][0])

                    def p2():
                        h2T = htp.tile([128, NCT, 512], BF16, tag="hT",
                                       name=f"h2T{rep_i}_{j}")
                        layernorm_transpose(j, st["r1s"], h2T, st["mv4b"],
                                            fused_rstd=True)
                        st["h2T"] = h2T
                        st["g"] = gp.tile([128, 16, 512], BF16, tag="g",
                                          name=f"g{rep_i}_{j}")

                    def fc_half(hh_):
                        for fh in (2 * hh_, 2 * hh_ + 1):
                            wfcq = wfcs.tile([128, NCT, 512], BF16,
                                             tag="wfcq")
                            nc.sync.dma_start(
                                wfcq, wfc_v[:, :, fh * 512:(fh + 1) * 512])
                            for fl in range(4):
                                ft = fh * 4 + fl
                                ups = ps_f.tile([128, 512], F32, tag="f")
                                for ct in range(NCT):
                                    nc.tensor.matmul(
                                        ups,
                                        wfcq[:, ct, fl * 128:(fl + 1) * 128],
                                        st["h2T"][:, ct, :],
                                        start=(ct == 0),
                                        stop=(ct == NCT - 1),
                                        skip_group_check=True)
                                if has_bfc:
                                    nc.scalar.activation(
                                        st["g"][:, ft, :], ups, AF.Gelu,
                                        bias=bfc_sb[:, ft:ft + 1])
                                else:
                                    nc.scalar.activation(
                                        st["g"][:, ft, :], ups, AF.Gelu)

                    def p3():
                        fc_half(0)

                    def p4():
                        fc_half(1)
                        if DEBUG_DUMP and j == 0:
                            nc.sync.dma_start(
                                dbg["d_g"][:].rearrange(
                                    "p (a b) -> p a b", a=16), st["g"])
                        st["ots"] = [otp.tile([128, C], BF16, tag="ot",
                                              name=f"ot{rep_i}_{j}_{t}")
                                     for t in range(4)]

                    def fcproj_half(nh):
                        wfph = wfps.tile([128, 16, 512], BF16, tag="wfph")
                        nc.sync.dma_start(
                            wfph, wfp_v[:, :, nh * 512:(nh + 1) * 512])
                        for tsub in range(4):
                            ops_ = ps_f.tile([128, 512], F32, tag="f")
                            for ft in range(16):
                                nc.tensor.matmul(
                                    ops_,
                                    st["g"][:, ft,
                                            tsub * 128:(tsub + 1) * 128],
                                    wfph[:, ft, :], start=(ft == 0),
                                    stop=(ft == 15), skip_group_check=True)
                            nc.vector.scalar_tensor_tensor(
                                out=st["ots"][tsub][:,
                                                    nh * 512:(nh + 1) * 512],
                                in0=st["r1s"][tsub][:,
                                                    nh * 512:(nh + 1) * 512],
                                scalar=0.5, in1=ops_,
                                op0=ALU.mult, op1=ALU.add)

                    def p5():
                        fcproj_half(0)

                    def p6():
                        fcproj_half(1)
                        for tsub in range(4):
                            nc.sync.dma_start(out_t[j * 4 + tsub],
                                              st["ots"][tsub])

                    return [p0, p1, p2, p3, p4, p5, p6]

                # zip: attention heads of chunk j interleave with FFN
                # pieces of chunk j-1 so PE always has dense work while
                # ACT grinds through the exp chain
                for j in range(NCH):
                    qT, ycon = phaseA_pre(j)
                    pieces = phaseB_pieces(j - 1) if j >= 1 else []
                    early, late = pieces[:5], pieces[5:]
                    for h in range(HPC):
                        attention_head(j, h, qT, ycon)
                        # delay B(j-1) pieces a couple heads so the
                        # AllGather they depend on has landed (PE queue is
                        # FIFO; a stalled piece blocks everything behind it)
                        if h >= 2 and h - 2 < len(early):
                            early[h - 2]()
                    allgather(j, ycon)
                    # fcproj of chunk j-1 lands here to cover the
                    # collective's latency before B(j) can start
                    for p in late:
                        p()
                for p in phaseB_pieces(NCH - 1):
                    p()

            with nc.allow_low_precision(reason="bf16 kernel by design"):
                for _rep in range(reps):
                    emit_block(_rep)

    nc.finalize()
    return nc


def _get_program(has_bqk, has_bv, has_bfc, reps=1):
    key = (has_bqk, has_bv, has_bfc, reps)
    if key not in _CACHED:
        _CACHED[key] = _build_program(has_bqk, has_bv, has_bfc, reps=reps)
    return _CACHED[key]


def _prep(x, ln1_w, ln1_b, ln2_w, ln2_b, w_attn, w_proj, w_fc, w_fc_proj,
          **unused):
    bf16 = ml_dtypes.bfloat16
    x = np.asarray(x, np.float32)
    ln1_w = np.asarray(ln1_w, np.float32)
    ln1_b = np.asarray(ln1_b, np.float32)
    ln2_w = np.asarray(ln2_w, np.float32)
    ln2_b = np.asarray(ln2_b, np.float32)
    w_attn = np.asarray(w_attn, np.float32)
    w_proj = np.asarray(w_proj, np.float32)
    w_fc = np.asarray(w_fc, np.float32)
    w_fc_proj = np.asarray(w_fc_proj, np.float32)

    scale = 1.0 / np.sqrt(D)
    in_maps = []
    bqk_all, bv_all, bfc_all = [], [], []
    for c in range(8):
        b, hh = c // 2, c % 2
        qr = slice(hh * FQ, (hh + 1) * FQ)
        kr = slice(C + hh * FQ, C + (hh + 1) * FQ)
        vr = slice(2 * C + hh * FQ, 2 * C + (hh + 1) * FQ)
        fr = slice(hh * FFH, (hh + 1) * FFH)
        wq = w_attn[qr] * ln1_w * scale
        wk = w_attn[kr] * ln1_w
        wv = w_attn[vr] * ln1_w
        bq = (w_attn[qr] @ ln1_b) * scale
        bk = w_attn[kr] @ ln1_b
        bv = w_attn[vr] @ ln1_b
        wfc_h = w_fc[fr] * ln2_w
        bfc = w_fc[fr] @ ln2_b
        m = {
            "xin": np.ascontiguousarray(x[b]),
            "wqk": np.ascontiguousarray(
                np.concatenate([wq.T, wk.T], axis=1)).astype(bf16),
            "wvt": np.ascontiguousarray(wv.T).astype(bf16),
            "wpt": np.ascontiguousarray(w_proj.T).astype(bf16),
            "wfct": np.ascontiguousarray(wfc_h.T).astype(bf16),
            "wfpt": np.ascontiguousarray(w_fc_proj[:, fr].T).astype(bf16),
        }
        bqk_all.append(np.stack([bq, bk]))
        bv_all.append(bv)
        bfc_all.append(bfc)
        in_maps.append(m)

    has_bqk = any(np.abs(a).max() > 0 for a in bqk_all)
    has_bv = any(np.abs(a).max() > 0 for a in bv_all)
    has_bfc = any(np.abs(a).max() > 0 for a in bfc_all)
    for c in range(8):
        if has_bqk:
            in_maps[c]["bqk"] = np.ascontiguousarray(bqk_all[c])
        if has_bv:
            in_maps[c]["bv"] = np.ascontiguousarray(bv_all[c])
        if has_bfc:
            in_maps[c]["bfc"] = np.ascontiguousarray(bfc_all[c])
    return in_maps, (has_bqk, has_bv, has_bfc)


def kernel(**inputs):
    in_maps, flags = _prep(**inputs)
    nc = _get_program(*flags)
    res = run_bass_kernel_spmd(nc, in_maps, list(range(8))).results

    outp = np.empty((B, T, C), np.float32)
    for b in range(B):
        outp[b] = (res[2 * b]["out"].astype(np.float32)
                   + res[2 * b + 1]["out"].astype(np.float32))
    return outp


# revision 44
# speedup vs baseline: 1.1658x; 1.1658x over previous
"""Trainium2 Bass kernel for a dense transformer block (nn_Block_25366076850386).

Sharding (8 cores): core c -> batch b = c//2, head-half hh = c%2.
Each core computes LN1+QKV+attention for its 8 heads over its full batch
row, AllGathers attention outputs (bf16) within the (2b, 2b+1) pair,
computes the full attention projection + residual, then FFN with the FF
dim split in half per core. Host sums the pair's partial outputs:
    out[b] = part[2b] + part[2b+1],  part = 0.5*r1 + ffn_half(r1)

v2: all GEMM operands in bf16 (tolerance 2e-2 allows it), core weights
resident in SBUF (loaded once), FFN weights streamed per chunk in
halves/quarters, chunk-pipelined emission so the AllGather overlaps the
next chunk's attention, batched softmax reciprocals, causal mask via
gpsimd affine_select on the exp() output (keeps DVE free), PE transposes
batched 4-per-PSUM-bank with one strided evacuation copy.
"""

import numpy as np
import ml_dtypes

import concourse.bass as bass
import concourse.mybir as mybir
from concourse import bacc
from concourse.tile import TileContext
from concourse.masks import make_identity
from concourse.bass_utils import run_bass_kernel_spmd

F32 = mybir.dt.float32
BF16 = mybir.dt.bfloat16
AF = mybir.ActivationFunctionType
ALU = mybir.AluOpType

B, T, C, H, D, FF = 4, 2048, 1024, 16, 64, 4096
HPC = H // 2          # heads per core = 8
FQ = HPC * D          # per-core q/k/v width = 512
FFH = FF // 2         # per-core FF width = 2048
NT = T // 128         # 16 token tiles
NCT = C // 128        # 8 channel tiles
NCH = T // 512        # 4 token chunks (512 each)
EPS = 1e-5

_CACHED = {}
DEBUG_DUMP = False


def _build_program(has_bqk: bool, has_bv: bool, has_bfc: bool, reps: int = 1):
    nc = bacc.Bacc()

    xin = nc.dram_tensor("xin", [T, C], F32, kind="ExternalInput")
    wqk = nc.dram_tensor("wqk", [C, 2 * FQ], BF16, kind="ExternalInput")
    wvt = nc.dram_tensor("wvt", [C, FQ], BF16, kind="ExternalInput")
    wpt = nc.dram_tensor("wpt", [C, C], BF16, kind="ExternalInput")
    wfct = nc.dram_tensor("wfct", [C, FFH], BF16, kind="ExternalInput")
    wfpt = nc.dram_tensor("wfpt", [FFH, C], BF16, kind="ExternalInput")
    out = nc.dram_tensor("out", [T, C], BF16, kind="ExternalOutput")
    dbg = {}
    if DEBUG_DUMP:
        for nm, shp in [("d_h1T", [128, NCT * 512]), ("d_qT", [128, 4 * 512]),
                        ("d_kT", [128, 4 * 512]),
                        ("d_vON", [128, 4 * HPC * (D + 1)]),
                        ("d_et", [128, 512]), ("d_ysb", [64, 512]),
                        ("d_rec", [1, 512]), ("d_ycon", [128, 4 * 512]),
                        ("d_yf", [128, 8 * 512]), ("d_r1", [128, C]),
                        ("d_g", [128, 16 * 512])]:
            dbg[nm] = nc.dram_tensor(nm, shp, BF16 if nm != "d_rec" else F32,
                                     kind="ExternalOutput")
    bqk_d = bv_d = bfc_d = None
    if has_bqk:
        bqk_d = nc.dram_tensor("bqk", [2, FQ], F32, kind="ExternalInput")
    if has_bv:
        bv_d = nc.dram_tensor("bv", [FQ], F32, kind="ExternalInput")
    if has_bfc:
        bfc_d = nc.dram_tensor("bfc", [FFH], F32, kind="ExternalInput")

    x_t = xin[:].rearrange("(nt p) c -> nt p c", p=128)
    wqk_r = wqk[:].rearrange("(ct p) f -> ct p f", p=128)
    wvt_r = wvt[:].rearrange("(ct p) f -> ct p f", p=128)
    wpt_r = wpt[:].rearrange("(g p) o -> g p o", p=128)
    wfc_v = wfct[:].rearrange("(ct p) f -> p ct f", p=128)    # [128, 8, 2048]
    wfp_v = wfpt[:].rearrange("(ft p) c -> p ft c", p=128)    # [128, 16, 1024]
    out_t = out[:].rearrange("(nt p) c -> nt p c", p=128)

    from contextlib import ExitStack
    with TileContext(nc) as tc:
        with ExitStack() as stack:
            def pool(name, bufs, space="SBUF"):
                return stack.enter_context(
                    tc.tile_pool(name=name, bufs=bufs, space=space))
            persist = pool("persist", 1)
            kvp = pool("kv", 1)
            lnp = pool("lnp", 2)
            xp = pool("xp", 2)
            xcp = pool("xcp", 4)
            qtp = pool("qtp", 1)
            htp = pool("htp", 2)
            etp = pool("etp", 3)
            ysp = pool("ysp", 6)
            dnp = pool("dnp", 1)
            ycp = pool("ycp", 1)
            yfp = pool("yfp", 1)
            x2p = pool("x2p", 2)
            r1p = pool("r1p", 4)
            gp = pool("gp", 1)
            otp = pool("otp", 4)
            wfcs = pool("wfcs", 2)
            wfps = pool("wfps", 1)
            ps_a = pool("ps_a", 2, "PSUM")
            ps_s = pool("ps_s", 2, "PSUM")
            ps_y = pool("ps_y", 2, "PSUM")
            ps_f = pool("ps_f", 2, "PSUM")
            dram = pool("dram", 1, "DRAM")
            # ---------- constants ----------
            ident = persist.tile([128, 128], BF16, tag="ident")
            make_identity(nc, ident)
            eps_sb = persist.tile([128, 1], F32, tag="eps")
            nc.vector.memset(eps_sb, EPS)
            ones64 = persist.tile([1, 64], F32, tag="ones64")
            nc.vector.memset(ones64, 1.0)

            # ---------- resident weights (bf16) ----------
            wqk_sb = persist.tile([128, NCT, 2 * FQ], BF16, tag="wqk")
            wv_sb = persist.tile([128, NCT, FQ], BF16, tag="wv")
            wp_sb = persist.tile([128, NCT, C], BF16, tag="wp")
            for ct in range(NCT):
                nc.sync.dma_start(wqk_sb[:, ct, :], wqk_r[ct])
                nc.sync.dma_start(wv_sb[:, ct, :], wvt_r[ct])
                nc.sync.dma_start(wp_sb[:, ct, :], wpt_r[ct])
            bqk_sb = bv_sb = bfc_sb = None
            if has_bqk:
                bqk_sb = persist.tile([128, 2, FQ // 128], F32, tag="bqk")
                nc.sync.dma_start(
                    bqk_sb, bqk_d[:].rearrange("q (g p) -> p q g", p=128))
            if has_bv:
                bv_sb = persist.tile([128, FQ // 128], F32, tag="bv")
                nc.sync.dma_start(
                    bv_sb, bv_d[:].rearrange("(g p) -> p g", p=128))
            if has_bfc:
                bfc_sb = persist.tile([128, FFH // 128], F32, tag="bfc")
                nc.sync.dma_start(
                    bfc_sb, bfc_d[:].rearrange("(g p) -> p g", p=128))

            def emit_block(rep_i):
                # persistent K^T [128(2h*64d), 4g, T] and V(+ones) tiles
                kT = kvp.tile([128, 4, T], BF16, tag="kT", name=f"kT{rep_i}")
                vON = kvp.tile([128, NT, HPC, D + 1], BF16, tag="vON",
                               name=f"vON{rep_i}")
                nc.gpsimd.memset(vON, 1.0)   # ones column; data cols redone
                agos = [None] * NCH
                r1_of = [None] * NCH
                ln_state = {}

                def layernorm_transpose(j, src_tiles, hT, mv4, fused_rstd):
                    """Normalize 4 token tiles of chunk j and transpose into
                    hT [128, NCT, 512].  src_tiles: 4 aps [128, C] (x fp32 for
                    LN1 two-pass; r1 bf16 for LN2 fused)."""
                    sd4 = lnp.tile([128, 4], F32, tag="sd4")
                    nc.scalar.activation(sd4, mv4[:, :, 1], AF.Sqrt,
                                         bias=eps_sb)
                    rstd4 = lnp.tile([128, 4], F32, tag="rstd4")
                    nc.vector.reciprocal(rstd4, sd4)
                    for tsub in range(4):
                        if fused_rstd:
                            h1 = xcp.tile([128, C], BF16, tag="xc")
                            nc.vector.tensor_scalar(
                                out=h1, in0=src_tiles[tsub],
                                scalar1=mv4[:, tsub, 0:1],
                                scalar2=rstd4[:, tsub:tsub + 1],
                                op0=ALU.subtract, op1=ALU.mult)
                        else:
                            h1 = src_tiles[tsub]
                            nc.vector.tensor_scalar(
                                out=h1, in0=h1,
                                scalar1=rstd4[:, tsub:tsub + 1], scalar2=None,
                                op0=ALU.mult)
                        tp = ps_a.tile([128, 1024], BF16, tag="ps")
                        for ct in range(NCT):
                            nc.tensor.matmul(
                                tp[:, ct * 128:(ct + 1) * 128],
                                h1[:, ct * 128:(ct + 1) * 128], ident,
                                is_transpose=True, skip_group_check=True)
                        nc.vector.tensor_copy(
                            hT[:, :, tsub * 128:(tsub + 1) * 128],
                            tp[:].rearrange("p (q f) -> p q f", q=NCT))

                def phaseA_pre(j):
                    # ---- LN1 (two-pass: xc = x - mu in bf16, then *rstd)
                    mv4 = lnp.tile([128, 4, 2], F32, tag="mv4")
                    xcs = []
                    for tsub in range(4):
                        it = j * 4 + tsub
                        xt = xp.tile([128, C], F32, tag="x")
                        nc.sync.dma_start(xt, x_t[it])
                        stats = lnp.tile([128, 2, 6], F32, tag="stats")
                        nc.vector.bn_stats(stats[:, 0, :], xt[:, 0:512])
                        nc.vector.bn_stats(stats[:, 1, :], xt[:, 512:1024])
                        nc.vector.bn_aggr(mv4[:, tsub, :], stats)
                        xc = xcp.tile([128, C], BF16, tag="xc")
                        nc.vector.tensor_scalar(
                            out=xc, in0=xt, scalar1=mv4[:, tsub, 0:1],
                            scalar2=None, op0=ALU.subtract)
                        xcs.append(xc)
                    h1T = htp.tile([128, NCT, 512], BF16, tag="hT",
                                   name=f"h1T{rep_i}_{j}")
                    layernorm_transpose(j, xcs, h1T, mv4, fused_rstd=False)
                    if DEBUG_DUMP and j == 0:
                        nc.sync.dma_start(
                            dbg["d_h1T"][:].rearrange(
                                "p (a b) -> p a b", a=NCT), h1T)

                    # ---- Q,K projections: out [f(128), 512t] per g
                    qT = qtp.tile([128, 4, 512], BF16, tag="qT",
                                  name=f"qT{rep_i}_{j}")
                    for g in range(8):
                        ps = ps_a.tile([128, 512], F32, tag="ps")
                        for ct in range(NCT):
                            nc.tensor.matmul(
                                ps, wqk_sb[:, ct, g * 128:(g + 1) * 128],
                                h1T[:, ct, :], start=(ct == 0),
                                stop=(ct == NCT - 1), skip_group_check=True)
                        if g < 4:
                            dst = qT[:, g, :]
                        else:
                            dst = kT[:, g - 4, j * 512:(j + 1) * 512]
                        if has_bqk:
                            nc.vector.tensor_scalar_add(
                                out=dst, in0=ps,
                                scalar1=bqk_sb[:, 0 if g < 4 else 1,
                                               g % 4:g % 4 + 1])
                        else:
                            nc.vector.tensor_copy(dst, ps)

                    if DEBUG_DUMP and j == 0:
                        nc.sync.dma_start(
                            dbg["d_qT"][:].rearrange(
                                "p (a b) -> p a b", a=4), qT)
                        nc.sync.dma_start(
                            dbg["d_kT"][:].rearrange(
                                "p (a b) -> p a b", a=4), kT[:, :, 0:512])
                    # ---- V projection: out [128t, 512f] per t-tile
                    for tsub in range(4):
                        it = j * 4 + tsub
                        ps = ps_a.tile([128, 512], F32, tag="ps")
                        for ct in range(NCT):
                            nc.tensor.matmul(
                                ps, h1T[:, ct, tsub * 128:(tsub + 1) * 128],
                                wv_sb[:, ct, :], start=(ct == 0),
                                stop=(ct == NCT - 1), skip_group_check=True)
                        nc.vector.tensor_copy(
                            vON[:, it, :, 0:D],
                            ps.rearrange("p (h d) -> p h d", h=HPC))

                    if DEBUG_DUMP and j == 0:
                        nc.sync.dma_start(
                            dbg["d_vON"][:].rearrange(
                                "p (a b c) -> p a b c", a=4, b=HPC),
                            vON[:, 0:4, :, :])
                    ycon = ycp.tile([128, 4, 512], BF16, tag="ycon",
                                    name=f"ycon{rep_i}_{j}")
                    return qT, ycon

                def attention_head(j, h, qT, ycon):
                    nkt = 4 * j + 4
                    g, poff = h // 2, (h % 2) * 64
                    yps = ps_y.tile([65, 512], F32, tag="y")
                    for kt in range(nkt):
                        sps = ps_s.tile([128, 512], F32, tag="s")
                        nc.tensor.matmul(
                            sps,
                            kT[poff:poff + 64, g, kt * 128:(kt + 1) * 128],
                            qT[poff:poff + 64, g, :],
                            start=True, stop=True, skip_group_check=True)
                        et = etp.tile([128, 512], BF16, tag="et")
                        nc.scalar.activation(et, sps, AF.Exp)
                        if kt >= 4 * j:
                            # zero the strictly-above-diagonal part
                            nc.gpsimd.affine_select(
                                out=et, in_=et, compare_op=ALU.is_ge,
                                fill=0.0, base=-128 * (kt - 4 * j),
                                pattern=[[1, 512]], channel_multiplier=-1)
                        if DEBUG_DUMP and j == 0 and h == 0 and kt == 0:
                            nc.sync.dma_start(dbg["d_et"][:], et)
                        nc.tensor.matmul(
                            yps, vON[:, kt, h, :], et, start=(kt == 0),
                            stop=(kt == nkt - 1), skip_group_check=True)
                    ysb = ysp.tile([64, 512], BF16, tag="ysb")
                    nc.vector.tensor_copy(ysb, yps[0:64, :])
                    den = dnp.tile([1, 512], F32, tag="den")
                    nc.scalar.activation(den, yps[64:65, :], AF.Copy)
                    rec = dnp.tile([1, 512], F32, tag="recf")
                    nc.vector.reciprocal_approx_fast(rec, den)
                    bcps = ps_a.tile([64, 512], F32, tag="ps")
                    nc.tensor.matmul(bcps, ones64, rec,
                                     start=True, stop=True,
                                     skip_group_check=True)
                    if DEBUG_DUMP and j == 0 and h == 0:
                        nc.sync.dma_start(dbg["d_ysb"][:], ysb)
                        nc.sync.dma_start(dbg["d_rec"][:], rec)
                    nc.vector.tensor_tensor(
                        out=ycon[poff:poff + 64, g, :],
                        in0=ysb, in1=bcps, op=ALU.mult)
                    if has_bv:
                        nc.vector.tensor_scalar_add(
                            out=ycon[poff:poff + 64, g, :],
                            in0=ycon[poff:poff + 64, g, :],
                            scalar1=bv_sb[poff:poff + 64, g:g + 1])

                def allgather(j, ycon):
                    if DEBUG_DUMP and j == 0:
                        nc.sync.dma_start(
                            dbg["d_ycon"][:].rearrange(
                                "p (a b) -> p a b", a=4), ycon)
                    agi = dram.tile([FQ, 512], BF16, tag=f"agi{rep_i}_{j}",
                                    name=f"agi{rep_i}_{j}")
                    nc.sync.dma_start(
                        agi[:].rearrange("(g p) q -> p g q", p=128), ycon)
                    ago = dram.tile([2 * FQ, 512], BF16,
                                    tag=f"ago{rep_i}_{j}",
                                    name=f"ago{rep_i}_{j}")
                    nc.gpsimd.collective_compute(
                        "AllGather", ALU.bypass,
                        replica_groups=[[0, 1], [2, 3], [4, 5], [6, 7]],
                        ins=[agi[:]], outs=[ago[:]])
                    agos[j] = ago

                def phaseB_pieces(j):
                    """FFN of chunk j as a list of closures for zipping."""
                    st = {}

                    def proj_tsub(tsub):
                        it = j * 4 + tsub
                        x2 = x2p.tile([128, C], F32, tag="x2")
                        nc.sync.dma_start(x2, x_t[it])
                        r1 = r1p.tile([128, C], BF16, tag="r1")
                        for nchk in range(2):
                            zps = ps_f.tile([128, 512], F32, tag="f")
                            for g8 in range(8):
                                nc.tensor.matmul(
                                    zps,
                                    st["yf"][:, g8,
                                             tsub * 128:(tsub + 1) * 128],
                                    wp_sb[:, g8, nchk * 512:(nchk + 1) * 512],
                                    start=(g8 == 0), stop=(g8 == 7),
                                    skip_group_check=True)
                            nc.vector.tensor_tensor(
                                out=r1[:, nchk * 512:(nchk + 1) * 512],
                                in0=zps,
                                in1=x2[:, nchk * 512:(nchk + 1) * 512],
                                op=ALU.add)
                        stats = lnp.tile([128, 2, 6], F32, tag="stats")
                        nc.vector.bn_stats(stats[:, 0, :], r1[:, 0:512])
                        nc.vector.bn_stats(stats[:, 1, :], r1[:, 512:1024])
                        nc.vector.bn_aggr(st["mv4b"][:, tsub, :], stats)
                        st["r1s"].append(r1)

                    def p0():
                        yf = yfp.tile([128, 8, 512], BF16, tag="yf",
                                      name=f"yf{rep_i}_{j}")
                        nc.sync.dma_start(
                            yf,
                            agos[j][:].rearrange("(g p) q -> p g q", p=128))
                        st["yf"] = yf
                        st["mv4b"] = lnp.tile([128, 4, 2], F32, tag="mv4",
                                      name=f"mv4b{rep_i}_{j}")
                        st["r1s"] = []
                        proj_tsub(0)
                        proj_tsub(1)

                    def p1():
                        proj_tsub(2)
                        proj_tsub(3)
                        r1_of[j] = st["r1s"]
                        if DEBUG_DUMP and j == 0:
                            nc.sync.dma_start(
                                dbg["d_yf"][:].rearrange(
                                    "p (a b) -> p a b", a=8), st["yf"])
                            nc.sync.dma_start(dbg["d_r1"][:], st["r1s"][0])

                    def p2():
                        h2T = htp.tile([128, NCT, 512], BF16, tag="hT",
                                       name=f"h2T{rep_i}_{j}")
                        layernorm_transpose(j, st["r1s"], h2T, st["mv4b"],
                                            fused_rstd=True)
                        st["h2T"] = h2T
                        st["g"] = gp.tile([128, 16, 512], BF16, tag="g",
                                          name=f"g{rep_i}_{j}")

                    def fc_half(hh_):
                        for fh in (2 * hh_, 2 * hh_ + 1):
                            wfcq = wfcs.tile([128, NCT, 512], BF16,
                                             tag="wfcq")
                            nc.sync.dma_start(
                                wfcq, wfc_v[:, :, fh * 512:(fh + 1) * 512])
                            for fl in range(4):
                                ft = fh * 4 + fl
                                ups = ps_f.tile([128, 512], F32, tag="f")
                                for ct in range(NCT):
                                    nc.tensor.matmul(
                                        ups,
                                        wfcq[:, ct, fl * 128:(fl + 1) * 128],
                                        st["h2T"][:, ct, :],
                                        start=(ct == 0),
                                        stop=(ct == NCT - 1),
                                        skip_group_check=True)
                                if has_bfc:
                                    nc.scalar.activation(
                                        st["g"][:, ft, :], ups, AF.Gelu,
                                        bias=bfc_sb[:, ft:ft + 1])
                                else:
                                    nc.scalar.activation(
                                        st["g"][:, ft, :], ups, AF.Gelu)

                    def p3():
                        fc_half(0)

                    def p4():
                        fc_half(1)
                        if DEBUG_DUMP and j == 0:
                            nc.sync.dma_start(
                                dbg["d_g"][:].rearrange(
                                    "p (a b) -> p a b", a=16), st["g"])
                        st["ots"] = [otp.tile([128, C], BF16, tag="ot",
                                              name=f"ot{rep_i}_{j}_{t}")
                                     for t in range(4)]

                    def fcproj_half(nh):
                        wfph = wfps.tile([128, 16, 512], BF16, tag="wfph")
                        nc.sync.dma_start(
                            wfph, wfp_v[:, :, nh * 512:(nh + 1) * 512])
                        for tsub in range(4):
                            ops_ = ps_f.tile([128, 512], F32, tag="f")
                            for ft in range(16):
                                nc.tensor.matmul(
                                    ops_,
                                    st["g"][:, ft,
                                            tsub * 128:(tsub + 1) * 128],
                                    wfph[:, ft, :], start=(ft == 0),
                                    stop=(ft == 15), skip_group_check=True)
                            nc.vector.scalar_tensor_tensor(
                                out=st["ots"][tsub][:,
                                                    nh * 512:(nh + 1) * 512],
                                in0=st["r1s"][tsub][:,
                                                    nh * 512:(nh + 1) * 512],
                                scalar=0.5, in1=ops_,
                                op0=ALU.mult, op1=ALU.add)

                    def p5():
                        fcproj_half(0)

                    def p6():
                        fcproj_half(1)
                        for tsub in range(4):
                            nc.sync.dma_start(out_t[j * 4 + tsub],
                                              st["ots"][tsub])

                    return [p0, p1, p2, p3, p4, p5, p6]

                # zip: attention heads of chunk j interleave with FFN
                # pieces of chunk j-1 so PE always has dense work while
                # ACT grinds through the exp chain
                for j in range(NCH):
                    qT, ycon = phaseA_pre(j)
                    pieces = phaseB_pieces(j - 1) if j >= 1 else []
                    early, late = pieces[:5], pieces[5:]
                    for h in range(HPC):
                        attention_head(j, h, qT, ycon)
                        # delay B(j-1) pieces a couple heads so the
                        # AllGather they depend on has landed (PE queue is
                        # FIFO; a stalled piece blocks everything behind it)
                        if h >= 2 and h - 2 < len(early):
                            early[h - 2]()
                    allgather(j, ycon)
                    # fcproj of chunk j-1 lands here to cover the
                    # collective's latency before B(j) can start
                    for p in late:
                        p()
                for p in phaseB_pieces(NCH - 1):
                    p()

            with nc.allow_low_precision(reason="bf16 kernel by design"):
                for _rep in range(reps):
                    emit_block(_rep)

    nc.finalize()
    return nc


def _get_program(has_bqk, has_bv, has_bfc, reps=1):
    key = (has_bqk, has_bv, has_bfc, reps)
    if key not in _CACHED:
        _CACHED[key] = _build_program(has_bqk, has_bv, has_bfc, reps=reps)
    return _CACHED[key]


def _prep(x, ln1_w, ln1_b, ln2_w, ln2_b, w_attn, w_proj, w_fc, w_fc_proj,
          **unused):
    bf16 = ml_dtypes.bfloat16
    x = np.asarray(x, np.float32)
    ln1_w = np.asarray(ln1_w, np.float32)
    ln1_b = np.asarray(ln1_b, np.float32)
    ln2_w = np.asarray(ln2_w, np.float32)
    ln2_b = np.asarray(ln2_b, np.float32)
    w_attn = np.asarray(w_attn, np.float32)
    w_proj = np.asarray(w_proj, np.float32)
    w_fc = np.asarray(w_fc, np.float32)
    w_fc_proj = np.asarray(w_fc_proj, np.float32)

    scale = 1.0 / np.sqrt(D)
    in_maps = []
    bqk_all, bv_all, bfc_all = [], [], []
    for c in range(8):
        b, hh = c // 2, c % 2
        qr = slice(hh * FQ, (hh + 1) * FQ)
        kr = slice(C + hh * FQ, C + (hh + 1) * FQ)
        vr = slice(2 * C + hh * FQ, 2 * C + (hh + 1) * FQ)
        fr = slice(hh * FFH, (hh + 1) * FFH)
        wq = w_attn[qr] * ln1_w * scale
        wk = w_attn[kr] * ln1_w
        wv = w_attn[vr] * ln1_w
        bq = (w_attn[qr] @ ln1_b) * scale
        bk = w_attn[kr] @ ln1_b
        bv = w_attn[vr] @ ln1_b
        wfc_h = w_fc[fr] * ln2_w
        bfc = w_fc[fr] @ ln2_b
        m = {
            "xin": np.ascontiguousarray(x[b]),
            "wqk": np.ascontiguousarray(
                np.concatenate([wq.T, wk.T], axis=1)).astype(bf16),
            "wvt": np.ascontiguousarray(wv.T).astype(bf16),
            "wpt": np.ascontiguousarray(w_proj.T).astype(bf16),
            "wfct": np.ascontiguousarray(wfc_h.T).astype(bf16),
            "wfpt": np.ascontiguousarray(w_fc_proj[:, fr].T).astype(bf16),
        }
        bqk_all.append(np.stack([bq, bk]))
        bv_all.append(bv)
        bfc_all.append(bfc)
        in_maps.append(m)

    has_bqk = any(np.abs(a).max() > 0 for a in bqk_all)
    has_bv = any(np.abs(a).max() > 0 for a in bv_all)
    has_bfc = any(np.abs(a).max() > 0 for a in bfc_all)
    for c in range(8):
        if has_bqk:
            in_maps[c]["bqk"] = np.ascontiguousarray(bqk_all[c])
        if has_bv:
            in_maps[c]["bv"] = np.ascontiguousarray(bv_all[c])
        if has_bfc:
            in_maps[c]["bfc"] = np.ascontiguousarray(bfc_all[c])
    return in_maps, (has_bqk, has_bv, has_bfc)


def kernel(**inputs):
    in_maps, flags = _prep(**inputs)
    nc = _get_program(*flags)
    res = run_bass_kernel_spmd(nc, in_maps, list(range(8))).results

    outp = np.empty((B, T, C), np.float32)
    for b in range(B):
        outp[b] = (res[2 * b]["out"].astype(np.float32)
                   + res[2 * b + 1]["out"].astype(np.float32))
    return outp
